# revision 1
# baseline (speedup 1.0000x reference)
"""Bass/Trainium2 kernel for NF4-dequant (QLoRA-style) SwiGLU MLP.

Computation (matches the bitsandbytes-NF4 reference):
    dq_i = nf4_quant_dequant(w_i)   (per-64-block absmax scaling)
    out  = dq3-proj( silu(x @ dq1^T) * (x @ dq2^T) )

Sharding: tensor-parallel over the ffn dim H=11008 across 8 cores.
H is split in 64-aligned shards of width [1408 x4, 1344 x4]; the 1344
shards are zero-padded to 1408 so every core runs the same program.
Each core computes a full [T, D] partial of the down-projection; the
host sums the 8 partials (the TP all-reduce).

On-device pipeline per core:
  phase 1: stream natural-layout weight tiles, per-64-block absmax ->
           reciprocal -> normalize -> 15-boundary bucketize (fused
           compare*delta tensor_scalar ops + int16 add chain) ->
           rescale -> PE-transpose -> DRAM scratch.
  phase 2: gate/up matmuls (PSUM f32 accum over D), SiLU on ACT,
           h = silu(gate)*up on GPSIMD, h tiles spilled to DRAM.
  phase 3: down-proj partial matmuls, PSUM evict on DVE, DMA out.

Emission interleaves tb0's gate/up with the w1/w2 dequant and w3's
dequant with the later token blocks so DVE and PE overlap.

Env knobs (compile-time): KERNEL_MM=bf16|f32r, KERNEL_CMP_INT16=0|1.
"""

import os
import sys

import numpy as np

if not os.path.isdir(os.path.join(os.path.dirname(os.path.abspath(__file__)), "concourse")):
    for _p in ("/opt/trn_rl_repo",):
        if os.path.isdir(_p) and _p not in sys.path:
            sys.path.insert(0, _p)

import concourse.bass as bass
import concourse.mybir as mybir
import concourse.tile as tile
from concourse import bacc
from concourse.bass_utils import run_bass_kernel_spmd
from concourse.masks import make_identity

F32 = mybir.dt.float32
F32R = mybir.dt.float32r
BF16 = mybir.dt.bfloat16
I16 = mybir.dt.int16
OP = mybir.AluOpType

NF4_CODE = np.array(
    [
        -1.0, -0.6961928009986877, -0.5250730514526367, -0.39491748809814453,
        -0.28444138169288635, -0.18477343022823334, -0.09105003625154495, 0.0,
        0.07958029955625534, 0.16093020141124725, 0.24611230194568634,
        0.33791524171829224, 0.44070982933044434, 0.5626170039176941,
        0.7229568362236023, 1.0,
    ],
    dtype=np.float32,
)
NF4_BOUNDS = ((NF4_CODE[:-1] + NF4_CODE[1:]) * np.float32(0.5)).astype(np.float32)
NF4_DELTAS = (NF4_CODE[1:] - NF4_CODE[:-1]).astype(np.float32)
VSCALE = 16384.0  # codes reconstructed as int16 / 2^14
NF4_IDELTAS = [int(x) for x in np.round(NF4_DELTAS * VSCALE)]
CSCALE = 32760.0  # int16 compare-domain scale (saturation-safe)
NF4_IBOUNDS = [int(np.floor(float(b) * CSCALE)) for b in NF4_BOUNDS]

BLK = 64

D = 4096
T_FULL = 4096
H_FULL = 11008
N_CORES = 8
HP = 1408
SHARD_W = [1408, 1408, 1408, 1408, 1344, 1344, 1344, 1344]
SHARD_START = [0, 1408, 2816, 4224, 5632, 6976, 8320, 9664]

NSUB = 512
KT = D // 128  # 32
HT = HP // 128  # 11

MM_MODE = os.environ.get("KERNEL_MM", "bf16")  # bf16 | f32r
CMP_INT16 = os.environ.get("KERNEL_CMP_INT16", "0") == "1"

if MM_MODE == "bf16":
    WDT = BF16  # matmul-operand storage dtype for dq scratch / x / h
    T_BLK = 1024
    DQ_CHUNK = 1024
    W3_CHUNKS = [(0, 640), (640, 768)]
else:
    # float32r: fp32-layout reduced-mantissa matmul dtype, full PE rate at
    # N>=256. Producers feeding the PE must round to the f32r grid.
    WDT = F32R
    T_BLK = 512
    DQ_CHUNK = 1024
    W3_CHUNKS = [(0, 640), (640, 768)]
NTB = T_FULL // T_BLK


class P:
    pass


def _emit_dequant_tile(nc, p, w_ap, row0, col0, cw, store_fn, identity):
    """Dequantize a [128, cw] natural tile; write transposed blocks."""
    nblk = cw // BLK

    wt = p.pw.tile([128, cw], F32, tag="wt", name="wt")
    nc.sync.dma_start(wt[:], w_ap[row0 : row0 + 128, col0 : col0 + cw])
    w3v = wt[:].rearrange("p (b i) -> p b i", i=BLK)

    amax = p.pa.tile([128, nblk], F32, tag="amax", name="amax")
    nc.vector.tensor_reduce(
        amax[:], w3v, axis=mybir.AxisListType.X, op=OP.max, apply_absolute_value=True
    )
    aclamp = p.pa.tile([128, nblk], F32, tag="aclamp", name="aclamp")
    nc.vector.tensor_scalar_max(aclamp[:], amax[:], 1e-35)
    recip = p.pa.tile([128, nblk], F32, tag="recip", name="recip")
    nc.vector.reciprocal(recip[:], aclamp[:])
    av = p.pa.tile([128, nblk], F32, tag="av", name="av")
    nc.vector.tensor_scalar_mul(av[:], amax[:], 1.0 / VSCALE)
    av_b = av[:].unsqueeze(2).broadcast_to([128, nblk, BLK])

    if CMP_INT16:
        rs = p.pa.tile([128, nblk], F32, tag="rs", name="rs")
        nc.vector.tensor_scalar_mul(rs[:], recip[:], CSCALE)
        r_b = rs[:].unsqueeze(2).broadcast_to([128, nblk, BLK])
        vn = p.pvn.tile([128, cw], I16, tag="vn", name="vn")
        bounds = NF4_IBOUNDS
    else:
        r_b = recip[:].unsqueeze(2).broadcast_to([128, nblk, BLK])
        vn = p.pvn.tile([128, cw], F32, tag="vn", name="vn")
        bounds = [float(b) for b in NF4_BOUNDS]
    vn3 = vn[:].rearrange("p (b i) -> p b i", i=BLK)
    nc.vector.tensor_tensor(vn3, w3v, r_b, OP.mult)

    def prod(j, out_ap):
        nc.vector.tensor_scalar(
            out_ap, vn[:], bounds[j], NF4_IDELTAS[j], OP.is_gt, OP.mult
        )

    acc = p.pchain.tile([128, cw], I16, tag="acc", name="acc")
    prod(0, acc[:])
    for j in range(1, 15):
        tmp = p.pprod.tile([128, cw], I16, tag="prod", name="tmp")
        prod(j, tmp[:])
        nc.vector.tensor_tensor(acc[:], acc[:], tmp[:], OP.add)

    dq = p.pdq.tile([128, cw], WDT, tag="dq", name="dq")
    dq3 = dq[:].rearrange("p (b i) -> p b i", i=BLK)
    acc3 = acc[:].rearrange("p (b i) -> p b i", i=BLK)
    # dq = (acc - 2^14) * (absmax / 2^14)
    nc.vector.scalar_tensor_tensor(dq3, acc3, -int(VSCALE), av_b, OP.add, OP.mult)

    for jb in range(cw // 128):
        ps = p.pps.tile([128, 128], WDT, tag="ps", name="tps")
        nc.tensor.transpose(ps[:], dq[:, jb * 128 : (jb + 1) * 128], identity[:])
        qt = p.pqt.tile([128, 128], WDT, tag="qt", name="qt")
        nc.scalar.copy(qt[:], ps[:])
        nc.gpsimd.dma_start(store_fn((col0 + jb * 128) // 128), qt[:])


def _build_program():
    nc = bacc.Bacc("TRN2", target_bir_lowering=False, debug=False, num_devices=N_CORES)

    xT = nc.dram_tensor("xT", [D, T_FULL], F32, kind="ExternalInput").ap()
    w1s = nc.dram_tensor("w1s", [HP, D], F32, kind="ExternalInput").ap()
    w2s = nc.dram_tensor("w2s", [HP, D], F32, kind="ExternalInput").ap()
    w3s = nc.dram_tensor("w3s", [D, HP], F32, kind="ExternalInput").ap()
    out = nc.dram_tensor("out", [T_FULL, D], F32, kind="ExternalOutput").ap()

    from contextlib import ExitStack

    with tile.TileContext(nc) as tc, ExitStack() as ctx:
        p = P()
        dram = ctx.enter_context(tc.tile_pool(name="dram", bufs=1, space="DRAM"))
        s1 = dram.tile([HT, 128, KT, 128], WDT)
        s2 = dram.tile([HT, 128, KT, 128], WDT)
        s3 = dram.tile([HT, 128, D], WDT)
        hTd = dram.tile([NTB, HT, 128, T_BLK], WDT)

        const = ctx.enter_context(tc.tile_pool(name="const", bufs=1))
        if MM_MODE == "bf16":
            identity = const.tile([128, 128], BF16)
            make_identity(nc, identity[:])
        else:
            identity_f = const.tile([128, 128], F32)
            make_identity(nc, identity_f[:])
            identity = const.tile([128, 128], F32R)
            nc.vector.tensor_copy(identity[:], identity_f[:])

        if MM_MODE == "bf16":
            pool_spec = [
                ("pw", 2), ("pa", 2), ("pvn", 2), ("pprod", 6), ("pchain", 4),
                ("pdq", 2), ("pqt", 4), ("px", 2), ("pxb", KT), ("pl", 2),
                ("pht", 3), ("psl", 4), ("pu", 4), ("phl", HT + 1), ("pr3", 3),
                ("pob", 4),
            ]
        else:
            pool_spec = [
                ("pw", 2), ("pa", 2), ("pvn", 2), ("pprod", 4), ("pchain", 4),
                ("pdq", 2), ("pqt", 2), ("px", 2), ("pxb", KT), ("pl", 3),
                ("pht", 3), ("psl", 2), ("phl", HT + 1), ("pr3", 3),
                ("pob", 2),
            ]
        for nm, bufs in pool_spec:
            setattr(p, nm, ctx.enter_context(tc.tile_pool(name=nm, bufs=bufs)))
        p.pps = ctx.enter_context(tc.tile_pool(name="pps", bufs=8, space="PSUM"))

        def dq_tile_w12(which, s, i):
            w_ap = w1s if which == 1 else w2s
            for ch in range(0, D, DQ_CHUNK):
                _emit_dequant_tile(
                    nc, p, w_ap, i * 128, ch, DQ_CHUNK,
                    lambda kt, i=i: s[i, :, kt, :], identity,
                )

        def load_x(tb):
            xk = []
            for k in range(KT):
                xf = p.px.tile([128, T_BLK], F32, tag="xf", name="xf")
                nc.sync.dma_start(
                    xf[:], xT[k * 128 : (k + 1) * 128, tb * T_BLK : (tb + 1) * T_BLK]
                )
                xb = p.pxb.tile([128, T_BLK], WDT, tag="xb", name="xb")
                if MM_MODE == "bf16":
                    nc.scalar.copy(xb[:], xf[:])
                else:
                    nc.vector.tensor_copy(xb[:], xf[:])  # rounds to f32r grid
                xk.append(xb)
            return xk

        # lhsT strips are loaded in segments (f32r strips are 16KB/partition
        # whole, too big to double-buffer)
        SEG = KT * 128 if MM_MODE == "bf16" else KT * 32

        def load_strip(s, h, tag):
            segs = []
            for k0 in range(0, KT * 128, SEG):
                seg = p.pl.tile([128, SEG], WDT, tag=tag, name=tag)
                nc.sync.dma_start(
                    seg[:],
                    s[h, :, k0 // 128 : (k0 + SEG) // 128, :].rearrange(
                        "p k i -> p (k i)"
                    ),
                )
                segs.append(seg)
            return segs

        def lhs_slice(segs, k):
            return segs[(k * 128) // SEG][
                :, (k * 128) % SEG : (k * 128) % SEG + 128
            ]

        def phase2_htile(tb, h, xk):
            l1 = load_strip(s1, h, "l1")
            l2 = load_strip(s2, h, "l2")
            ht = p.pht.tile([128, T_BLK], WDT, tag="ht", name="ht")
            for c in range(T_BLK // NSUB):
                pg = p.pps.tile([128, NSUB], F32, tag="ps", name="pg")
                pu = p.pps.tile([128, NSUB], F32, tag="ps", name="pu")
                for k in range(KT):
                    nc.tensor.matmul(
                        pg[:],
                        lhs_slice(l1, k),
                        xk[k][:, c * NSUB : (c + 1) * NSUB],
                        start=(k == 0),
                        stop=(k == KT - 1),
                    )
                for k in range(KT):
                    nc.tensor.matmul(
                        pu[:],
                        lhs_slice(l2, k),
                        xk[k][:, c * NSUB : (c + 1) * NSUB],
                        start=(k == 0),
                        stop=(k == KT - 1),
                    )
                if MM_MODE == "bf16":
                    sl = p.psl.tile([128, NSUB], BF16, tag="sl", name="sl")
                    nc.scalar.activation(
                        sl[:], pg[:], mybir.ActivationFunctionType.Silu
                    )
                    ue = p.pu.tile([128, NSUB], BF16, tag="ue", name="ue")
                    nc.scalar.copy(ue[:], pu[:])
                    nc.gpsimd.tensor_tensor(
                        ht[:, c * NSUB : (c + 1) * NSUB], sl[:], ue[:], OP.mult
                    )
                else:
                    sl = p.psl.tile([128, NSUB], F32, tag="sl", name="sl")
                    nc.scalar.activation(
                        sl[:], pg[:], mybir.ActivationFunctionType.Silu
                    )
                    # rounds to f32r on write
                    nc.vector.tensor_tensor(
                        ht[:, c * NSUB : (c + 1) * NSUB], sl[:], pu[:], OP.mult
                    )
            nc.gpsimd.dma_start(hTd[tb, h, :, :], ht[:])

        def phase3(tb):
            strips = []
            for k in range(HT):
                hl = p.phl.tile([128, T_BLK], WDT, tag="hl", name="hl")
                nc.sync.dma_start(hl[:], hTd[tb, k, :, :])
                strips.append(hl)
            # process tt in halves of 4 PSUM banks so consecutive (dc, half)
            # iterations pipeline through the 8-bank pool instead of
            # serializing on all-8-bank evictions at each dc boundary
            TTH = max(1, T_BLK // 128 // 2)
            for dc in range(D // NSUB):
                for th in range(0, T_BLK // 128, TTH):
                    po = [
                        p.pps.tile([128, NSUB], F32, tag="ps", name=f"po{tt}")
                        for tt in range(th, th + TTH)
                    ]
                    for k in range(HT):
                        r3 = p.pr3.tile([128, NSUB], WDT, tag="r3", name="r3")
                        nc.sync.dma_start(
                            r3[:], s3[k, :, dc * NSUB : (dc + 1) * NSUB]
                        )
                        for i, tt in enumerate(range(th, th + TTH)):
                            nc.tensor.matmul(
                                po[i][:],
                                strips[k][:, tt * 128 : (tt + 1) * 128],
                                r3[:],
                                start=(k == 0), stop=(k == HT - 1),
                            )
                    for i, tt in enumerate(range(th, th + TTH)):
                        ob = p.pob.tile([128, NSUB], F32, tag="ob", name="ob")
                        nc.vector.tensor_copy(ob[:], po[i][:])
                        nc.gpsimd.dma_start(
                            out[
                                tb * T_BLK + tt * 128 : tb * T_BLK + (tt + 1) * 128,
                                dc * NSUB : (dc + 1) * NSUB,
                            ],
                            ob[:],
                        )

        w3_work = [
            (i, ch, cw) for i in range(KT) for (ch, cw) in W3_CHUNKS
        ]
        w3_iter = iter(w3_work)

        def emit_w3(n):
            for _ in range(n):
                item = next(w3_iter, None)
                if item is None:
                    return
                i, ch, cw = item
                _emit_dequant_tile(
                    nc, p, w3s, i * 128, ch, cw,
                    lambda hb, i=i: s3[hb, :, i * 128 : (i + 1) * 128], identity,
                )

        xk0 = load_x(0)
        for i in range(HT):
            dq_tile_w12(1, s1, i)
            dq_tile_w12(2, s2, i)
            phase2_htile(0, i, xk0)
        n_slots = (NTB - 1) * HT
        per_slot = -(-len(w3_work) // n_slots) if n_slots else len(w3_work)
        for tb in range(1, NTB):
            xk = load_x(tb)
            for h in range(HT):
                emit_w3(per_slot)
                phase2_htile(tb, h, xk)
        emit_w3(len(w3_work))
        for tb in range(NTB):
            phase3(tb)

    nc.compile()
    return nc


_CACHED_NC = None
LAST_RESULTS = None


def _shard_inputs(x, w1, w2, w3):
    xT = np.ascontiguousarray(x.reshape(T_FULL, D).T, dtype=np.float32)
    in_maps = []
    for c in range(N_CORES):
        s, w = SHARD_START[c], SHARD_W[c]
        w1c = np.zeros((HP, D), dtype=np.float32)
        w1c[:w] = w1[s : s + w]
        w2c = np.zeros((HP, D), dtype=np.float32)
        w2c[:w] = w2[s : s + w]
        w3c = np.zeros((D, HP), dtype=np.float32)
        w3c[:, :w] = w3[:, s : s + w]
        in_maps.append({"xT": xT, "w1s": w1c, "w2s": w2c, "w3s": w3c})
    return in_maps


def kernel(x, w1, w2, w3):
    global _CACHED_NC, LAST_RESULTS
    assert x.shape == (2, 2048, D) and w1.shape == (H_FULL, D)
    if _CACHED_NC is None:
        _CACHED_NC = _build_program()
    in_maps = _shard_inputs(x, w1, w2, w3)
    res = run_bass_kernel_spmd(
        _CACHED_NC,
        in_maps,
        core_ids=list(range(N_CORES)),
        trace=os.environ.get("KERNEL_TRACE", "") == "1",
    )
    LAST_RESULTS = res
    acc = res.results[0]["out"].astype(np.float32).copy()
    for c in range(1, N_CORES):
        acc += res.results[c]["out"]
    return acc.reshape(2, 2048, D).astype(np.float32)



# revision 8
# speedup vs baseline: 1.5317x; 1.5317x over previous
"""Bass/Trainium2 kernel for NF4-dequant (QLoRA-style) SwiGLU MLP.

Computation (matches the bitsandbytes-NF4 reference):
    dq_i = nf4_quant_dequant(w_i)   (per-64-block absmax scaling)
    out  = dq3-proj( silu(x @ dq1^T) * (x @ dq2^T) )

Sharding: tensor-parallel over the ffn dim H=11008 across 8 cores.
H is split in 64-aligned shards of width [1408 x4, 1344 x4]; the 1344
shards are zero-padded to 1408 so every core runs the same program.
Each core computes a full [T, D] partial of the down-projection; the
host sums the 8 partials (the TP all-reduce).

Dequant runs on custom DVE (vector-engine) micro-ops:
  vn   = round_i16(w * (32760/absmax))            (stock tensor_tensor)
  m    = sum_j (vn > IB_j) - 7.5, j=0..14         (5 custom ops: A1 takes
                                                   4 bounds via C3-spill,
                                                   3x ACC, A5 adds bias)
  code ~= c0 + poly7(m)                           (custom H1+H2; deg-7 LSQ
                                                   fit of the NF4 codebook
                                                   in m, max err 2.2e-3)
  dq   = (h' + c0) * absmax                       (stock scalar_tensor_tensor)
All matmul operands are fp16 (full PE rate, more mantissa than bf16).
dq tiles are transposed to lhsT layout with the DMA crossbar
(dma_start_transpose), not the PE.

Schedule per core:
  phase A (h-strips in groups of 2): dequant w1/w2 strips -> xbar ->
    for each 512-token chunk: stream x (pre-cast fp16) once per group,
    gate/up matmul chains, silu on ACT, h-mult on GPSIMD, h chunks
    DMA'd to DRAM.  The DVE dequants group g+1 while PE works group g.
  phase B (four 1024-d quarters, double-buffered): w3 dequanted
    just-in-time and xbar'd into SBUF; down-proj matmuls consume h
    strips streamed back; PSUM evicted on ACT; partials DMA'd out.
"""

import os
import sys

import numpy as np

if not os.path.isdir(os.path.join(os.path.dirname(os.path.abspath(__file__)), "concourse")):
    for _p in ("/opt/trn_rl_repo",):
        if os.path.isdir(_p) and _p not in sys.path:
            sys.path.insert(0, _p)

import concourse.bass as bass
import concourse.mybir as mybir
import concourse.tile as tile
from concourse import bacc
from concourse.bass_utils import run_bass_kernel_spmd

F32 = mybir.dt.float32
BF16 = mybir.dt.bfloat16
FP16 = mybir.dt.float16
I16 = mybir.dt.int16
OP = mybir.AluOpType

# ---------------- NF4 constants + reconstruction fit ----------------
NF4_CODE = np.array([
    -1.0, -0.6961928009986877, -0.5250730514526367, -0.39491748809814453,
    -0.28444138169288635, -0.18477343022823334, -0.09105003625154495, 0.0,
    0.07958029955625534, 0.16093020141124725, 0.24611230194568634,
    0.33791524171829224, 0.44070982933044434, 0.5626170039176941,
    0.7229568362236023, 1.0], dtype=np.float64)
NF4_BOUNDS = (NF4_CODE[:-1] + NF4_CODE[1:]) * 0.5
CSCALE = 32760.0
IB = [float(np.floor(b * CSCALE) + 0.5) for b in NF4_BOUNDS]

_n = np.arange(16)
_mp = (_n - 7.5) / 7.5
_c, *_ = np.linalg.lstsq(np.stack([_mp**k for k in range(8)], -1), NF4_CODE, rcond=None)
H_C0 = float(_c[0])
S_M = float(_c[1] * 2.0 / 15.0)
AT = [float(_c[6] / _c[1]**6), float(_c[4] / _c[1]**4), float(_c[2] / _c[1]**2)]
BT = [float(_c[7] / _c[1]**7), float(_c[5] / _c[1]**5), float(_c[3] / _c[1]**3)]

D = 4096
T_FULL = 4096
H_FULL = 11008
N_CORES = 8
HP = 1408
SHARD_W = [1408, 1408, 1408, 1408, 1344, 1344, 1344, 1344]
SHARD_START = [0, 1408, 2816, 4224, 5632, 6976, 8320, 9664]

KT = D // 128          # 32 d-tiles
HT = HP // 128         # 11 h-strips
NSUB = 512             # psum free width
NTC = T_FULL // NSUB   # 8 token chunks in phase A
GRP = 2                # h-strips per phase-A group (x reuse factor)
DQRT = 1024            # phase B d-quarter
BLK = 64
HW = 2048              # dequant working width


# ---------------- custom DVE op registration ----------------
def _register_nf4_ops():
    from concourse.dve_spec import Spec, Src0, Src1, C0, C1, C2, C3, sq, lower
    from concourse.dve_spec import _has_src1, _spill_c3_to_src1
    from concourse.dve_uop import DveOpSpec
    import concourse.dve_ops as dops

    def add_op(name, body, ref, spill=False):
        for o in dops.OPS:
            if o.name == name:
                return o
        if spill:
            body = _spill_c3_to_src1(body)
        spec = Spec(body=body, reference=ref)
        row = max(dops._SUB_OPCODE_FOR_NAME.values()) + 1
        assert row < 0x20, "DVE opcode rows exhausted"
        uops = lower(spec, ver="v3")
        sha = DveOpSpec(name=name, opcode=row, uops=uops,
                        rd1_en=_has_src1(spec)).sha("v3")
        op = dops.DveOp(name, spec, subdim=False, uops_sha={"v3": sha})
        dops.OPS.append(op)
        dops.CUSTOM_DVE_SPECS[name] = spec
        dops._SUB_OPCODE_FOR_NAME[name] = row
        return op

    a1 = add_op(
        "NF4A1",
        (Src0 > C0) + (Src0 > C1) + (Src0 > C2) + (Src0 > C3),
        lambda in0, in1, s0, s1, imm2: (
            (in0 > s0).astype(np.float32) + (in0 > s1).astype(np.float32)
            + (in0 > imm2).astype(np.float32) + (in0 > in1[..., :1]).astype(np.float32)
        ).astype(np.float32),
        spill=True,
    )
    acc = add_op(
        "NF4ACC",
        Src1 + (Src0 > C0) + (Src0 > C1) + (Src0 > C2),
        lambda in0, in1, s0, s1, imm2: (
            in1 + (in0 > s0) + (in0 > s1) + (in0 > imm2)
        ).astype(np.float32),
    )
    a5 = add_op(
        "NF4A5",
        Src1 + (Src0 > C0) + (Src0 > C1) + C2,
        lambda in0, in1, s0, s1, imm2: (
            in1 + (in0 > s0) + (in0 > s1) + imm2
        ).astype(np.float32),
    )
    u = sq(Src0)
    h1 = add_op(
        "NF4H1",
        ((C3 * u + C1) * u + C2) * u + Src0,
        lambda in0, in1, s0, s1, imm2: (
            ((in1[..., :1] * in0 * in0 + s1) * in0 * in0 + imm2) * in0 * in0 + in0
        ).astype(np.float32),
        spill=True,
    )
    u2 = sq(Src1)
    h2 = add_op(
        "NF4H2",
        Src0 + (((C0 * u2 + C1) * u2 + C2) * u2) * Src1,
        lambda in0, in1, s0, s1, imm2: (
            in0 + (((s0 * in1 * in1 + s1) * in1 * in1 + imm2) * in1 * in1) * in1
        ).astype(np.float32),
    )
    return a1, acc, a5, h1, h2


class P:
    pass


def _build_program():
    OPA1, OPACC, OPA5, OPH1, OPH2 = _register_nf4_ops()
    nc = bacc.Bacc("TRN2", target_bir_lowering=False, debug=False, num_devices=N_CORES)

    xT = nc.dram_tensor("xT", [D, T_FULL], F32, kind="ExternalInput").ap()
    w1s = nc.dram_tensor("w1s", [HP, D], F32, kind="ExternalInput").ap()
    w2s = nc.dram_tensor("w2s", [HP, D], F32, kind="ExternalInput").ap()
    w3s = nc.dram_tensor("w3s", [D, HP], F32, kind="ExternalInput").ap()
    out = nc.dram_tensor("out", [T_FULL, D], F32, kind="ExternalOutput").ap()

    from contextlib import ExitStack

    with tile.TileContext(nc) as tc, ExitStack() as ctx:
        p = P()
        dram = ctx.enter_context(tc.tile_pool(name="dram", bufs=1, space="DRAM"))
        xTb = dram.tile([D, T_FULL], FP16)          # pre-cast fp16 activations
        hTd = dram.tile([HT, 128, T_FULL], FP16)    # h strips (h-part, t-free)

        const = ctx.enter_context(tc.tile_pool(name="const", bufs=1))
        spill_a1 = const.tile([128, 1], F32)
        nc.vector.memset(spill_a1[:], IB[3])
        spill_h1 = const.tile([128, 1], F32)
        nc.vector.memset(spill_h1[:], AT[0])

        pool_spec = [
            ("pw", 2),      # raw w f32 [128, 2048] (also x-precast staging)
            ("pa", 2),      # absmax smalls
            ("pvn", 1),     # vn i16
            ("pcnt", 1),    # count ping/pong fp16 (tags ca, cb)
            ("pdq", 2),     # dq fp16 (also x-precast fp16 staging)
            ("plt", 4),     # lhsT strips [128, 32, 128] fp16 (tags l1, l2)
            ("px", 4),      # x stream quarters [128, 8, 512] fp16
            ("psl", 3),     # silu / up / precast [128, 512] fp16
            ("pht", 3),     # h chunks [128, 512] fp16
            ("pr3", 2),     # phase B w3 lhsT-T quarters [128, 8, 11, 128] fp16
            ("phs", 13),    # phase B h stream [128, 512] fp16
            ("pob", 2),     # out evict f32 [128, 512]
        ]
        for nm, bufs in pool_spec:
            setattr(p, nm, ctx.enter_context(tc.tile_pool(name=nm, bufs=bufs)))
        p.pps = ctx.enter_context(tc.tile_pool(name="pps", bufs=8, space="PSUM"))

        def dequant_tile(w_ap, row0, col0, nb):
            """Dequant [128, nb*64] at (row0, col0) of w_ap -> fp16 tile."""
            cw = nb * BLK
            wt = p.pw.tile([128, HW], F32, tag="wt", name="wt")
            nc.sync.dma_start(wt[:, :cw], w_ap[row0:row0 + 128, col0:col0 + cw])
            wv = wt[:, :cw].rearrange("p (b i) -> p b i", i=BLK)
            amax = p.pa.tile([128, HW // BLK], F32, tag="amax", name="amax")
            nc.vector.tensor_reduce(amax[:, :nb], wv, axis=mybir.AxisListType.X,
                                    op=OP.max, apply_absolute_value=True)
            acl = p.pa.tile([128, HW // BLK], F32, tag="acl", name="acl")
            nc.vector.tensor_scalar_max(acl[:, :nb], amax[:, :nb], 1e-20)
            rcs = p.pa.tile([128, HW // BLK], F32, tag="rcs", name="rcs")
            nc.vector.reciprocal(rcs[:, :nb], acl[:, :nb])
            rcc = p.pa.tile([128, HW // BLK], F32, tag="rcc", name="rcc")
            nc.vector.tensor_scalar_mul(rcc[:, :nb], rcs[:, :nb], CSCALE)
            av = p.pa.tile([128, HW // BLK], FP16, tag="av", name="av")
            nc.vector.tensor_copy(av[:, :nb], amax[:, :nb])

            vn = p.pvn.tile([128, HW], I16, tag="vn", name="vn")
            vn3 = vn[:, :cw].rearrange("p (b i) -> p b i", i=BLK)
            nc.vector.tensor_tensor(
                vn3, wv, rcc[:, :nb].unsqueeze(2).broadcast_to([128, nb, BLK]), OP.mult)

            ca = p.pcnt.tile([128, HW], FP16, tag="ca", name="ca")
            cb = p.pcnt.tile([128, HW], FP16, tag="cb", name="cb")
            nc.vector._custom_dve(OPA1, out=ca[:, :cw], in0=vn[:, :cw],
                                  in1=spill_a1[:], s0=IB[0], s1=IB[1], imm2=IB[2])
            nc.vector._custom_dve(OPACC, out=cb[:, :cw], in0=vn[:, :cw], in1=ca[:, :cw],
                                  s0=IB[4], s1=IB[5], imm2=IB[6])
            nc.vector._custom_dve(OPACC, out=ca[:, :cw], in0=vn[:, :cw], in1=cb[:, :cw],
                                  s0=IB[7], s1=IB[8], imm2=IB[9])
            nc.vector._custom_dve(OPACC, out=cb[:, :cw], in0=vn[:, :cw], in1=ca[:, :cw],
                                  s0=IB[10], s1=IB[11], imm2=IB[12])
            nc.vector._custom_dve(OPA5, out=ca[:, :cw], in0=vn[:, :cw], in1=cb[:, :cw],
                                  s0=IB[13], s1=IB[14], imm2=-7.5)
            # ca holds m = idx - 7.5; scale to mt = m * S_M (|mt| <= 0.62)
            mt = p.pcnt.tile([128, HW], FP16, tag="mt", name="mt")
            nc.vector.tensor_scalar_mul(mt[:, :cw], ca[:, :cw], S_M)
            nc.vector._custom_dve(OPH1, out=cb[:, :cw], in0=mt[:, :cw],
                                  in1=spill_h1[:], s0=0.0, s1=AT[1], imm2=AT[2])
            nc.vector._custom_dve(OPH2, out=cb[:, :cw], in0=cb[:, :cw], in1=mt[:, :cw],
                                  s0=BT[0], s1=BT[1], imm2=BT[2])
            dq = p.pdq.tile([128, HW], FP16, tag="dq", name="dq")
            dq3 = dq[:, :cw].rearrange("p (b i) -> p b i", i=BLK)
            cb3 = cb[:, :cw].rearrange("p (b i) -> p b i", i=BLK)
            nc.vector.scalar_tensor_tensor(
                dq3, cb3, H_C0,
                av[:, :nb].unsqueeze(2).broadcast_to([128, nb, BLK]), OP.add, OP.mult)
            return dq

        # ---------------- x precast: f32 xT -> fp16 xTb (t-chunk major) ----
        for tc in range(NTC):
            for kh in range(8):  # 4 k-tiles per staging tile
                xf = p.pw.tile([128, HW], F32, tag="wt", name="xf")
                xf4 = xf[:].rearrange("p (k t) -> p k t", t=NSUB)
                nc.sync.dma_start(
                    xf4,
                    xT.rearrange("(k p) t -> p k t", p=128)[
                        :, kh * 4:(kh + 1) * 4, tc * NSUB:(tc + 1) * NSUB])
                xh = p.pdq.tile([128, HW], FP16, tag="dq", name="xh")
                nc.scalar.copy(xh[:], xf[:])
                nc.gpsimd.dma_start(
                    xTb.rearrange("(k p) t -> p k t", p=128)[
                        :, kh * 4:(kh + 1) * 4, tc * NSUB:(tc + 1) * NSUB],
                    xh[:].rearrange("p (k t) -> p k t", t=NSUB))

        # ---------------- phase A: gate/up + h ----------------
        def dequant_strip(w_ap, i, tag):
            lt = p.plt.tile([128, KT, 128], FP16, tag=tag, name=tag)
            for half in range(2):
                dq = dequant_tile(w_ap, i * 128, half * HW, HW // BLK)
                nc.sync.dma_start_transpose(
                    lt[:, half * (KT // 2):(half + 1) * (KT // 2), :], dq[:])
            return lt

        groups = [list(range(g, min(g + GRP, HT))) for g in range(0, HT, GRP)]
        for grp in groups:
            lts = []
            for i in grp:
                l1 = dequant_strip(w1s, i, "l1")
                l2 = dequant_strip(w2s, i, "l2")
                lts.append((l1, l2))
            for tc in range(NTC):
                xq = []
                for q in range(4):
                    xk = p.px.tile([128, 8, NSUB], FP16, tag="xk", name="xk")
                    nc.sync.dma_start(
                        xk[:], xTb.rearrange("(k p) t -> p k t", p=128)[
                            :, q * 8:(q + 1) * 8, tc * NSUB:(tc + 1) * NSUB])
                    xq.append(xk)
                for si, i in enumerate(grp):
                    l1, l2 = lts[si]
                    pg = p.pps.tile([128, NSUB], F32, tag="ps", name="pg")
                    pu = p.pps.tile([128, NSUB], F32, tag="ps", name="pu")
                    for k in range(KT):
                        xs = xq[k // 8][:, k % 8, :]
                        nc.tensor.matmul(pg[:], l1[:, k, :], xs,
                                         start=(k == 0), stop=(k == KT - 1))
                    for k in range(KT):
                        xs = xq[k // 8][:, k % 8, :]
                        nc.tensor.matmul(pu[:], l2[:, k, :], xs,
                                         start=(k == 0), stop=(k == KT - 1))
                    sl = p.psl.tile([128, NSUB], FP16, tag="sl", name="sl")
                    nc.scalar.activation(sl[:], pg[:],
                                         mybir.ActivationFunctionType.Silu)
                    ue = p.psl.tile([128, NSUB], FP16, tag="ue", name="ue")
                    nc.scalar.copy(ue[:], pu[:])
                    htc = p.pht.tile([128, NSUB], FP16, tag="htc", name="htc")
                    nc.gpsimd.tensor_tensor(htc[:], sl[:], ue[:], OP.mult)
                    nc.gpsimd.dma_start(
                        hTd[i, :, tc * NSUB:(tc + 1) * NSUB], htc[:])

        # ---------------- phase B: down-projection ----------------
        for qd in range(D // DQRT):
            d0 = qd * DQRT
            r3 = p.pr3.tile([128, DQRT // 128, HT, 128], FP16, tag="r3", name="r3")
            for dt in range(DQRT // 128):
                dq = dequant_tile(w3s, d0 + dt * 128, 0, HP // BLK)
                nc.sync.dma_start_transpose(r3[:, dt, :, :], dq[:, :HP])
            for tg in range(T_FULL // NSUB):
                hs = []
                for kh in range(HT):
                    h = p.phs.tile([128, NSUB], FP16, tag="hs", name="hs")
                    nc.sync.dma_start(
                        h[:], hTd[kh, :, tg * NSUB:(tg + 1) * NSUB])
                    hs.append(h)
                for dcq in range(DQRT // NSUB):
                    for tt in range(NSUB // 128):
                        po = p.pps.tile([128, NSUB], F32, tag="ps", name="po")
                        for kh in range(HT):
                            nc.tensor.matmul(
                                po[:], hs[kh][:, tt * 128:(tt + 1) * 128],
                                r3[:, 4 * dcq:4 * (dcq + 1), kh, :],
                                start=(kh == 0), stop=(kh == HT - 1))
                        ob = p.pob.tile([128, NSUB], F32, tag="ob", name="ob")
                        nc.scalar.copy(ob[:], po[:])
                        nc.gpsimd.dma_start(
                            out[tg * NSUB + tt * 128:tg * NSUB + (tt + 1) * 128,
                                d0 + dcq * NSUB:d0 + (dcq + 1) * NSUB],
                            ob[:])

    nc.compile()
    return nc


_CACHED_NC = None
LAST_RESULTS = None


def _shard_inputs(x, w1, w2, w3):
    xT = np.ascontiguousarray(x.reshape(T_FULL, D).T, dtype=np.float32)
    in_maps = []
    for c in range(N_CORES):
        s, w = SHARD_START[c], SHARD_W[c]
        w1c = np.zeros((HP, D), dtype=np.float32)
        w1c[:w] = w1[s:s + w]
        w2c = np.zeros((HP, D), dtype=np.float32)
        w2c[:w] = w2[s:s + w]
        w3c = np.zeros((D, HP), dtype=np.float32)
        w3c[:, :w] = w3[:, s:s + w]
        in_maps.append({"xT": xT, "w1s": w1c, "w2s": w2c, "w3s": w3c})
    return in_maps


def kernel(x, w1, w2, w3):
    global _CACHED_NC, LAST_RESULTS
    assert x.shape == (2, 2048, D) and w1.shape == (H_FULL, D)
    if _CACHED_NC is None:
        _CACHED_NC = _build_program()
    in_maps = _shard_inputs(x, w1, w2, w3)
    res = run_bass_kernel_spmd(
        _CACHED_NC,
        in_maps,
        core_ids=list(range(N_CORES)),
        trace=os.environ.get("KERNEL_TRACE", "") == "1",
    )
    LAST_RESULTS = res
    acc = res.results[0]["out"].astype(np.float32).copy()
    for c in range(1, N_CORES):
        acc += res.results[c]["out"]
    return acc.reshape(2, 2048, D).astype(np.float32)
